# revision 46
# baseline (speedup 1.0000x reference)
"""Multi-head causal attention (B=4, S=2048, D=1024, H=16) on 8 TRN2 NeuronCores.

Sharding: core c -> (batch c//2, head-group c%2 of 8 heads = 512 d_model cols).
Each core:
  - projects Q/K/V for its head slice (bf16 matmuls, fp32 accum)
  - causal attention for its 8 heads over the full sequence, computed with
    scores transposed ([keys, q]) so exp(scores)^T feeds the A@V matmul as the
    moving operand; the stationary is [V(64) | ones(64)] per head so ctx lands
    on psum rows 0-63 and the softmax sums replicated on rows 64-127 in one
    accumulation group
  - normalization: sums -> SBUF, reciprocal, multiply straight out of PSUM
  - partial out-projection ctx^T @ Wo[rows-of-its-heads]  (no bias)
Host: out[b] = partial[2b] + partial[2b+1] + bo.

DMA loads are batched into a few large descriptors (host pre-packs weights)
since DMA triggers serialize on the Sync sequencer at ~600ns each; x chunks
are prefetched one chunk ahead.  Each chunk's own kT/V projections run as PE
filler inside the chunk (deadline-paced before the diagonal needs them), so
late chunks - where exp dominates - keep PE filler work available.
"""

import numpy as np
import ml_dtypes
from contextlib import ExitStack

import concourse.bass as bass
import concourse.tile as tile
from concourse import bacc, mybir
from concourse.bass_utils import run_bass_kernel_spmd

F32 = mybir.dt.float32
BF16 = mybir.dt.bfloat16
I32 = mybir.dt.int32
EXP = mybir.ActivationFunctionType.Exp

# Schraudolph exp bit-trick in bf16 bit-layout: exp(s*SCALE) ~=
#   bitcast_bf16(i16(s * SK16 + SB16)); ~2% RMS error.  One DVE
# tensor_scalar per tile offloads softmax exp from the saturated Scalar
# engine; the ctx matmul reads the int16 tile bitcast as bf16.
SK16 = 0.125 * 1.4426950408889634 * 128.0
SB16 = 127.0 * 128.0 - 366393.0 / 65536.0 + 0.5

N_CORES = 8
S = 2048          # sequence length
D = 1024          # d_model
HL = 8            # heads per core
HD = 64           # head dim
DL = HL * HD      # local d_model slice = 512
SCALE = 1.0 / 8.0  # 1/sqrt(HD)

NQC = S // 512    # 4 q chunks of 512
NDT = D // 128    # 8 d_model(in) tiles
NMT = DL // 128   # 4 local dout tiles (head pairs)

_compiled = None  # cached (nc,) so repeated kernel() calls skip rebuild


def _build():
    nc = bacc.Bacc("TRN2", target_bir_lowering=False, debug=False,
                   num_devices=N_CORES)

    # host-packed inputs (see _shard):
    #   wqkv: [3*D, DL] bf16 (Wq|Wk|Wv rows), wo: [DL, D] bf16
    #   bqk:  [128, 8] f32 (bq m-tiles in cols 0-3, bk in cols 4-7)
    #   bvb:  [128, DL] f32 (bv broadcast over partitions)
    #   xqt/xkt/xvt: [D, S] bf16 (x transposed)
    xq_ap = nc.dram_tensor("xqt", [D, S], BF16, kind="ExternalInput").ap()
    xk_ap = nc.dram_tensor("xkt", [D, S], BF16, kind="ExternalInput").ap()
    xv_ap = nc.dram_tensor("xvt", [D, S], BF16, kind="ExternalInput").ap()
    wqkv_ap = nc.dram_tensor("wqkv", [3 * D, DL], BF16, kind="ExternalInput").ap()
    bqk_ap = nc.dram_tensor("bqk", [128, 2 * NMT], F32, kind="ExternalInput").ap()
    bvb_ap = nc.dram_tensor("bvb", [128, DL], F32, kind="ExternalInput").ap()
    wo_ap = nc.dram_tensor("wo", [DL, D], BF16, kind="ExternalInput").ap()
    out_ap = nc.dram_tensor("out", [S, D], BF16, kind="ExternalOutput").ap()

    with tile.TileContext(nc) as tc, ExitStack() as ctx:
        wpool = ctx.enter_context(tc.tile_pool(name="weights", bufs=1))
        xt_pool = ctx.enter_context(tc.tile_pool(name="xt", bufs=7))
        qkv_pool = ctx.enter_context(tc.tile_pool(name="qkv", bufs=1))
        exp_pool = ctx.enter_context(tc.tile_pool(name="expt", bufs=6))
        sch_pool = ctx.enter_context(tc.tile_pool(name="sch", bufs=3))
        norm_pool = ctx.enter_context(tc.tile_pool(name="norm", bufs=2))
        outst_pool = ctx.enter_context(tc.tile_pool(name="outst", bufs=2))
        # PSUM: scores pool 2 x [128,1024] (4 banks) + proj/outproj pool
        # 2 x [128,512] (2 banks, so filler bias-adds never block the scores
        # ring) + ctx accumulators 2 x [128,512] (2 banks)
        psum_big = ctx.enter_context(tc.tile_pool(name="ps_big", bufs=2, space="PSUM"))
        psum_proj = ctx.enter_context(tc.tile_pool(name="ps_proj", bufs=2, space="PSUM"))
        psum_ctx = ctx.enter_context(tc.tile_pool(name="ps_ctx", bufs=2, space="PSUM"))

        # ---- batched weight / bias loads ----
        # wqkv -> [128, 3, 8, 512]: (p,(i,d,c)) <- dram row 1024*i+128*d+p.
        # Issued as 3 DMAs interleaved with the x chunk-0 loads (emitted just
        # below) so the first projection's inputs (wq + xq0) complete first.
        wqkv_sb = wpool.tile([128, 3, NDT, DL], BF16, tag="wqkv")
        wqkv_src = wqkv_ap.rearrange("(i d p) c -> p i d c", i=3, d=NDT)
        wq_sb = [wqkv_sb[:, 0, d, :] for d in range(NDT)]
        wk_sb = [wqkv_sb[:, 1, d, :] for d in range(NDT)]
        wv_sb = [wqkv_sb[:, 2, d, :] for d in range(NDT)]

        # ---- x^T chunk load: one DMA per (input, chunk) -> [128, 8, 512] ----
        def load_xt_chunk(x_ap, qc, nm):
            t = xt_pool.tile([128, NDT, 512], BF16, tag="xt", name=f"{nm}xt{qc}")
            nc.sync.dma_start(
                t[:],
                x_ap[:, 512 * qc:512 * (qc + 1)].rearrange(
                    "(d p) c -> p d c", d=NDT))
            return t

        # chunk-0 loads.  The SDMA engines round-robin all queued transfers at
        # packet granularity, so everything queued at t=0 finishes together.
        # To give the first projection's inputs (wq + xq0) the full bandwidth,
        # the remaining startup loads are issued from the Scalar engine's
        # HWDGE queue behind a dummy op that depends on the wq DMA.
        nc.sync.dma_start(wqkv_sb[:, 0], wqkv_src[:, 0])
        xq_c0 = load_xt_chunk(xq_ap, 0, "q")

        dummy_sb = wpool.tile([1, 1], F32, tag="dummy")
        nc.scalar.activation(dummy_sb[:], wqkv_sb[0:1, 0, 0, 0:1],
                             mybir.ActivationFunctionType.Copy)

        def load_xt_chunk_act(x_ap, qc, nm):
            t = xt_pool.tile([128, NDT, 512], BF16, tag="xt", name=f"{nm}xt{qc}")
            nc.scalar.dma_start(
                t[:],
                x_ap[:, 512 * qc:512 * (qc + 1)].rearrange(
                    "(d p) c -> p d c", d=NDT))
            return t

        bqk_sb = wpool.tile([128, 2 * NMT], F32, tag="bqk")
        nc.scalar.dma_start(bqk_sb[:], bqk_ap)
        bvb_sb = wpool.tile([128, DL], F32, tag="bvb")
        nc.scalar.dma_start(bvb_sb[:], bvb_ap)
        nc.scalar.dma_start(wqkv_sb[:, 1], wqkv_src[:, 1])
        xk_c0 = load_xt_chunk_act(xk_ap, 0, "k")
        nc.scalar.dma_start(wqkv_sb[:, 2], wqkv_src[:, 2])
        xv_c0 = load_xt_chunk_act(xv_ap, 0, "v")

        # wo -> [128, 4, 1024]
        wo_sb4 = wpool.tile([128, NMT, D], BF16, tag="wo")
        nc.scalar.dma_start(wo_sb4[:], wo_ap.rearrange("(d p) c -> p d c", d=NMT))
        wo_sb = [wo_sb4[:, d, :] for d in range(NMT)]

        # qT/kT: [DL, S] bf16 stored as NMT tiles [128, S]
        qT = [qkv_pool.tile([128, S], BF16, tag=f"qT{m}", name=f"qT{m}") for m in range(NMT)]
        kT = [qkv_pool.tile([128, S], BF16, tag=f"kT{m}", name=f"kT{m}") for m in range(NMT)]

        def proj_chunk(xt, w_sb, bcol, res, qc, m):
            ps = psum_proj.tile([128, 512], F32, tag="proj", name="ps")
            for d in range(NDT):
                nc.tensor.matmul(
                    ps[:], w_sb[d][:, 128 * m:128 * (m + 1)],
                    xt[:, d, :],
                    start=(d == 0), stop=(d == NDT - 1))
            nc.vector.tensor_scalar_add(
                res[m][:, 512 * qc:512 * (qc + 1)], ps[:],
                bqk_sb[:, bcol:bcol + 1])

        # v_aug: per seq-tile [128, HL, 2*HD] bf16; per head [V(64) | ones(64)]
        # so the ctx matmul (M=128, one accumulation group) leaves ctx on psum
        # rows 0-63 and the softmax sums replicated on rows 64-127.  The ones
        # halves are memset once upfront (DVE is idle during the initial DMA).
        v_aug = [qkv_pool.tile([128, HL, 2 * HD], BF16, tag=f"va{st}",
                               name=f"va{st}") for st in range(S // 128)]
        for st in range(S // 128):
            nc.vector.memset(v_aug[st][:, :, HD:2 * HD], 1.0)

        def v_chunk(xt, qc, sti):
            st = 4 * qc + sti
            ps = psum_proj.tile([128, 512], F32, tag="proj", name="ps")
            for d in range(NDT):
                nc.tensor.matmul(ps[:], xt[:, d, 128 * sti:128 * (sti + 1)],
                                 wv_sb[d][:], start=(d == 0), stop=(d == NDT - 1))
            nc.vector.tensor_add(
                v_aug[st][:, :, 0:HD],
                ps[:].rearrange("p (h c) -> p h c", h=HL),
                bvb_sb[:].rearrange("p (h c) -> p h c", h=HL))

        # ---- out projection ----
        ctxT = [qkv_pool.tile([128, S], BF16, tag=f"ctxT{m}", name=f"ctxT{m}") for m in range(NMT)]

        def emit_outproj(qt):
            # evacuation via ScalarE Copy (DVE is busier than ScalarE here)
            ot = outst_pool.tile([128, 1024], BF16, tag="ot", name="ot")
            for n in range(2):
                po_ps = psum_proj.tile([128, 512], F32, tag="proj", name="po_ps")
                for d in range(NMT):
                    nc.tensor.matmul(
                        po_ps[:],
                        ctxT[d][:, 128 * qt:128 * (qt + 1)],
                        wo_sb[d][:, 512 * n:512 * (n + 1)],
                        start=(d == 0), stop=(d == NMT - 1))
                nc.scalar.activation(ot[:, 512 * n:512 * (n + 1)], po_ps[:],
                                     mybir.ActivationFunctionType.Copy)
            nc.sync.dma_start(out_ap[128 * qt:128 * (qt + 1), :], ot[:])

        # ---- chunk 0 projections upfront (diagonal starts at kt=0) ----
        for m in range(NMT):
            proj_chunk(xq_c0, wq_sb, m, qT, 0, m)
        for m in range(NMT):
            proj_chunk(xk_c0, wk_sb, NMT + m, kT, 0, m)
        for sti in range(4):
            v_chunk(xv_c0, 0, sti)

        # x prefetch state: chunk qc's x tiles are loaded during chunk qc-1
        xk_next = xv_next = None

        # ---- attention per q-chunk ----
        for qc in range(NQC):
            xk_cur, xv_cur = xk_next, xv_next
            nkt = 4 * (qc + 1)
            ndiag = 4 * qc
            G = (HL // 2) * nkt  # global iteration count for this chunk

            # filler list: (deadline_g, closure); sorted by deadline.
            # Deadlines are emission-order iterations g = hp*nkt + kt by which
            # the closure must have been emitted (data-dependency order).
            items = []
            BIG = 10 ** 6
            if qc > 0:
                # this chunk's v tiles (st >= 1): ctx(kt=ndiag+s) emitted at
                # python iteration ndiag+s+4 (pend depth); keep margin 2
                for s in range(1, 4):
                    items.append((min(ndiag + s + 2, nkt),
                                  lambda s=s, x=xv_cur: v_chunk(x, qc, s)))
                # this chunk's kT m-groups (m >= 1): scores(kt=ndiag) of pair
                # m emitted at g = m*nkt + ndiag; margin 2
                for m in range(1, NMT):
                    items.append((m * nkt + ndiag - 2,
                                  lambda m=m, x=xk_cur: proj_chunk(x, wk_sb, NMT + m, kT, qc, m)))
            if qc + 1 < NQC:
                xq_c = load_xt_chunk(xq_ap, qc + 1, "q")
                xk_next = load_xt_chunk(xk_ap, qc + 1, "k")
                xv_next = load_xt_chunk(xv_ap, qc + 1, "v")
                for m in range(NMT):
                    items.append((BIG, lambda m=m, x=xq_c, q=qc + 1: proj_chunk(x, wq_sb, m, qT, q, m)))
                # hoist next chunk's first kT / v group into this chunk
                items.append((BIG, lambda x=xk_next, q=qc + 1: proj_chunk(x, wk_sb, NMT + 0, kT, q, 0)))
                items.append((BIG, lambda x=xv_next, q=qc + 1: v_chunk(x, q, 0)))
            if qc > 0:
                items += [(BIG, lambda qt=4 * (qc - 1) + j: emit_outproj(qt))
                          for j in range(4)]
            items.sort(key=lambda it: it[0])
            nit = len(items)
            ndone = 0

            for hp in range(HL // 2):
                heads = (2 * hp, 2 * hp + 1)
                ctx_ps = {h: psum_ctx.tile([128, 512], F32, tag="ctx",
                                           name=f"ctx{h}") for h in heads}

                def emit_scores_exp(kt):
                    qs = max(0, 128 * kt - 512 * qc)
                    sc_ps = psum_big.tile([128, 1024], F32, tag="big", name="sc")
                    for i in range(2):
                        po = 64 * i
                        nc.tensor.matmul(
                            sc_ps[:, 512 * i + qs:512 * (i + 1)],
                            kT[hp][po:po + HD, 128 * kt:128 * (kt + 1)],
                            qT[hp][po:po + HD, 512 * qc + qs:512 * (qc + 1)],
                            start=True, stop=True)
                    # late chunks are Scalar(exp)-bound: offload a slice of
                    # non-diagonal tiles to DVE via the exp bit-trick
                    offload = (qc == 2 and kt % 4 == 3 and kt < ndiag) or \
                              (qc == 3 and kt % 3 == 2 and kt < ndiag)
                    if offload:
                        it = sch_pool.tile([128, 1024], mybir.dt.int16,
                                           tag="it", name="it")
                        nc.vector.tensor_scalar(
                            it[:], sc_ps[:], SK16, SB16,
                            mybir.AluOpType.mult, mybir.AluOpType.add)
                        return it[:].bitcast(BF16)
                    et = exp_pool.tile([128, 1024], BF16, tag="et", name="et")
                    nc.scalar.activation(et[:, qs:1024], sc_ps[:, qs:1024],
                                         EXP, scale=SCALE)
                    return et

                def emit_ctx(kt, et):
                    qs = max(0, 128 * kt - 512 * qc)
                    diag = ndiag <= kt
                    for i, h in enumerate(heads):
                        if diag:  # mask k>q in the diagonal 128x128 block
                            nc.gpsimd.affine_select(
                                out=et[:, 512 * i + qs:512 * i + qs + 128],
                                in_=et[:, 512 * i + qs:512 * i + qs + 128],
                                compare_op=mybir.AluOpType.is_ge, fill=0.0,
                                base=0, pattern=[[1, 128]], channel_multiplier=-1)
                        nc.tensor.matmul(
                            ctx_ps[h][:, qs:512],
                            v_aug[kt][:, h, :],
                            et[:, 512 * i + qs:512 * (i + 1)],
                            start=(kt == 0), stop=(kt == nkt - 1))

                # software pipeline: scores/exp run ahead of ctx
                pend = []
                for kt in range(nkt):
                    pend.append((kt, emit_scores_exp(kt)))
                    if len(pend) > 4:
                        emit_ctx(*pend.pop(0))
                    # fillers: emit when due (deadline) or by uniform pacing
                    g = hp * nkt + kt + 1
                    want = (nit * g) // G
                    while ndone < nit and (ndone < want or items[ndone][0] <= g):
                        items[ndone][1]()
                        ndone += 1
                # force items due within this pair before draining ctx
                lim = (hp + 1) * nkt
                while ndone < nit and items[ndone][0] <= lim:
                    items[ndone][1]()
                    ndone += 1
                for pn in pend:
                    emit_ctx(*pn)

                # normalization: replicated sums (psum rows 64-127) -> SBUF
                # base 0 (the custom recip op requires base-0 operands),
                # reciprocal, then multiply straight out of PSUM
                for h in heads:
                    po = 64 * (h % 2)
                    sums = norm_pool.tile([HD, 512], F32, tag="sums", name="sums")
                    nc.vector.tensor_copy(sums[:], ctx_ps[h][64:128, :])
                    recip = norm_pool.tile([HD, 512], F32, tag="recip", name="recip")
                    nc.vector.reciprocal_approx_fast(recip[:], sums[:])
                    nc.vector.tensor_mul(
                        ctxT[hp][po:po + HD, 512 * qc:512 * (qc + 1)],
                        ctx_ps[h][0:HD, :], recip[:])

            while ndone < nit:
                items[ndone][1]()
                ndone += 1

        for qt in range(4 * (NQC - 1), 4 * NQC):
            emit_outproj(qt)

    nc.compile()
    return nc


def _shard(inputs):
    in_maps = []
    for c in range(N_CORES):
        b, g = c // 2, c % 2
        sl = slice(512 * g, 512 * (g + 1))
        wqkv = np.concatenate([
            inputs["Wq"][:, sl], inputs["Wk"][:, sl], inputs["Wv"][:, sl]],
            axis=0).astype(ml_dtypes.bfloat16)
        bqk = np.empty((128, 8), np.float32)
        for m in range(4):
            bqk[:, m] = inputs["bq"][sl][128 * m:128 * (m + 1)]
            bqk[:, 4 + m] = inputs["bk"][sl][128 * m:128 * (m + 1)]
        in_maps.append({
            "xqt": np.ascontiguousarray(inputs["inputs_q"][b].T.astype(ml_dtypes.bfloat16)),
            "xkt": np.ascontiguousarray(inputs["inputs_k"][b].T.astype(ml_dtypes.bfloat16)),
            "xvt": np.ascontiguousarray(inputs["inputs_v"][b].T.astype(ml_dtypes.bfloat16)),
            "wqkv": np.ascontiguousarray(wqkv),
            "bqk": bqk,
            "bvb": np.ascontiguousarray(
                np.broadcast_to(inputs["bv"][sl], (128, 512))).astype(np.float32),
            "wo": np.ascontiguousarray(inputs["Wo"][sl, :].astype(ml_dtypes.bfloat16)),
        })
    return in_maps


def kernel(**inputs):
    global _compiled
    inputs = {k: np.asarray(v, dtype=np.float32) for k, v in inputs.items()}
    if _compiled is None:
        _compiled = _build()
    nc = _compiled
    in_maps = _shard(inputs)
    res = run_bass_kernel_spmd(nc, in_maps, list(range(N_CORES)),
                               trace=bool(int(__import__("os").environ.get("BASS_TRACE", "0"))))
    kernel.last_results = res
    B = 4
    out = np.empty((B, S, D), np.float32)
    for b in range(B):
        out[b] = (res.results[2 * b]["out"].astype(np.float32)
                  + res.results[2 * b + 1]["out"].astype(np.float32))
    out += inputs["bo"][None, None, :]
    return out


# revision 47
# speedup vs baseline: 1.0097x; 1.0097x over previous
"""Multi-head causal attention (B=4, S=2048, D=1024, H=16) on 8 TRN2 NeuronCores.

Sharding: core c -> (batch c//2, head-group c%2 of 8 heads = 512 d_model cols).
Each core:
  - projects Q/K/V for its head slice (bf16 matmuls, fp32 accum)
  - causal attention for its 8 heads over the full sequence, computed with
    scores transposed ([keys, q]) so exp(scores)^T feeds the A@V matmul as the
    moving operand; the stationary is [V(64) | ones(64)] per head so ctx lands
    on psum rows 0-63 and the softmax sums replicated on rows 64-127 in one
    accumulation group
  - normalization: sums -> SBUF, reciprocal, multiply straight out of PSUM
  - partial out-projection ctx^T @ Wo[rows-of-its-heads]  (no bias)
Host: out[b] = partial[2b] + partial[2b+1] + bo.

DMA loads are batched into a few large descriptors (host pre-packs weights)
since DMA triggers serialize on the Sync sequencer at ~600ns each; x chunks
are prefetched one chunk ahead.  Each chunk's own kT/V projections run as PE
filler inside the chunk (deadline-paced before the diagonal needs them), so
late chunks - where exp dominates - keep PE filler work available.
"""

import numpy as np
import ml_dtypes
from contextlib import ExitStack

import concourse.bass as bass
import concourse.tile as tile
from concourse import bacc, mybir
from concourse.bass_utils import run_bass_kernel_spmd

F32 = mybir.dt.float32
BF16 = mybir.dt.bfloat16
I32 = mybir.dt.int32
EXP = mybir.ActivationFunctionType.Exp

# Schraudolph exp bit-trick in bf16 bit-layout: exp(s*SCALE) ~=
#   bitcast_bf16(i16(s * SK16 + SB16)); ~2% RMS error.  One DVE
# tensor_scalar per tile offloads softmax exp from the saturated Scalar
# engine; the ctx matmul reads the int16 tile bitcast as bf16.
SK16 = 0.125 * 1.4426950408889634 * 128.0
SB16 = 127.0 * 128.0 - 366393.0 / 65536.0 + 0.5

N_CORES = 8
S = 2048          # sequence length
D = 1024          # d_model
HL = 8            # heads per core
HD = 64           # head dim
DL = HL * HD      # local d_model slice = 512
SCALE = 1.0 / 8.0  # 1/sqrt(HD)

NQC = S // 512    # 4 q chunks of 512
NDT = D // 128    # 8 d_model(in) tiles
NMT = DL // 128   # 4 local dout tiles (head pairs)

_compiled = None  # cached (nc,) so repeated kernel() calls skip rebuild


def _build():
    nc = bacc.Bacc("TRN2", target_bir_lowering=False, debug=False,
                   num_devices=N_CORES)

    # host-packed inputs (see _shard):
    #   wqkv: [3*D, DL] bf16 (Wq|Wk|Wv rows), wo: [DL, D] bf16
    #   bqk:  [128, 8] f32 (bq m-tiles in cols 0-3, bk in cols 4-7)
    #   bvb:  [128, DL] f32 (bv broadcast over partitions)
    #   xqt/xkt/xvt: [D, S] bf16 (x transposed)
    xq_ap = nc.dram_tensor("xqt", [D, S], BF16, kind="ExternalInput").ap()
    xk_ap = nc.dram_tensor("xkt", [D, S], BF16, kind="ExternalInput").ap()
    xv_ap = nc.dram_tensor("xvt", [D, S], BF16, kind="ExternalInput").ap()
    wqkv_ap = nc.dram_tensor("wqkv", [3 * D, DL], BF16, kind="ExternalInput").ap()
    bqk_ap = nc.dram_tensor("bqk", [128, 2 * NMT], F32, kind="ExternalInput").ap()
    bvb_ap = nc.dram_tensor("bvb", [128, DL], F32, kind="ExternalInput").ap()
    wo_ap = nc.dram_tensor("wo", [DL, D], BF16, kind="ExternalInput").ap()
    out_ap = nc.dram_tensor("out", [S, D], BF16, kind="ExternalOutput").ap()

    with tile.TileContext(nc) as tc, ExitStack() as ctx:
        wpool = ctx.enter_context(tc.tile_pool(name="weights", bufs=1))
        xt_pool = ctx.enter_context(tc.tile_pool(name="xt", bufs=7))
        qkv_pool = ctx.enter_context(tc.tile_pool(name="qkv", bufs=1))
        exp_pool = ctx.enter_context(tc.tile_pool(name="expt", bufs=6))
        sch_pool = ctx.enter_context(tc.tile_pool(name="sch", bufs=3))
        norm_pool = ctx.enter_context(tc.tile_pool(name="norm", bufs=2))
        outst_pool = ctx.enter_context(tc.tile_pool(name="outst", bufs=2))
        # PSUM: scores pool 2 x [128,1024] (4 banks) + proj/outproj pool
        # 2 x [128,512] (2 banks, so filler bias-adds never block the scores
        # ring) + ctx accumulators 2 x [128,512] (2 banks)
        psum_big = ctx.enter_context(tc.tile_pool(name="ps_big", bufs=2, space="PSUM"))
        psum_proj = ctx.enter_context(tc.tile_pool(name="ps_proj", bufs=2, space="PSUM"))
        psum_ctx = ctx.enter_context(tc.tile_pool(name="ps_ctx", bufs=2, space="PSUM"))

        # ---- batched weight / bias loads ----
        # wqkv -> [128, 3, 8, 512]: (p,(i,d,c)) <- dram row 1024*i+128*d+p.
        # Issued as 3 DMAs interleaved with the x chunk-0 loads (emitted just
        # below) so the first projection's inputs (wq + xq0) complete first.
        wqkv_sb = wpool.tile([128, 3, NDT, DL], BF16, tag="wqkv")
        wqkv_src = wqkv_ap.rearrange("(i d p) c -> p i d c", i=3, d=NDT)
        wq_sb = [wqkv_sb[:, 0, d, :] for d in range(NDT)]
        wk_sb = [wqkv_sb[:, 1, d, :] for d in range(NDT)]
        wv_sb = [wqkv_sb[:, 2, d, :] for d in range(NDT)]

        # ---- x^T chunk load: one DMA per (input, chunk) -> [128, 8, 512] ----
        def load_xt_chunk(x_ap, qc, nm):
            t = xt_pool.tile([128, NDT, 512], BF16, tag="xt", name=f"{nm}xt{qc}")
            nc.sync.dma_start(
                t[:],
                x_ap[:, 512 * qc:512 * (qc + 1)].rearrange(
                    "(d p) c -> p d c", d=NDT))
            return t

        # chunk-0 loads.  The SDMA engines round-robin all queued transfers at
        # packet granularity, so everything queued at t=0 finishes together.
        # To give the first projection's inputs (wq + xq0) the full bandwidth,
        # the remaining startup loads are issued from the Scalar engine's
        # HWDGE queue behind a dummy op that depends on the wq DMA.
        nc.sync.dma_start(wqkv_sb[:, 0], wqkv_src[:, 0])
        xq_c0 = load_xt_chunk(xq_ap, 0, "q")

        dummy_sb = wpool.tile([1, 1], F32, tag="dummy")
        nc.scalar.activation(dummy_sb[:], xq_c0[0:1, 0, 0:1],
                             mybir.ActivationFunctionType.Copy)

        def load_xt_chunk_act(x_ap, qc, nm):
            t = xt_pool.tile([128, NDT, 512], BF16, tag="xt", name=f"{nm}xt{qc}")
            nc.scalar.dma_start(
                t[:],
                x_ap[:, 512 * qc:512 * (qc + 1)].rearrange(
                    "(d p) c -> p d c", d=NDT))
            return t

        bqk_sb = wpool.tile([128, 2 * NMT], F32, tag="bqk")
        nc.scalar.dma_start(bqk_sb[:], bqk_ap)
        bvb_sb = wpool.tile([128, DL], F32, tag="bvb")
        nc.scalar.dma_start(bvb_sb[:], bvb_ap)
        nc.scalar.dma_start(wqkv_sb[:, 1], wqkv_src[:, 1])
        xk_c0 = load_xt_chunk_act(xk_ap, 0, "k")
        nc.scalar.dma_start(wqkv_sb[:, 2], wqkv_src[:, 2])
        xv_c0 = load_xt_chunk_act(xv_ap, 0, "v")

        # wo -> [128, 4, 1024]
        wo_sb4 = wpool.tile([128, NMT, D], BF16, tag="wo")
        nc.scalar.dma_start(wo_sb4[:], wo_ap.rearrange("(d p) c -> p d c", d=NMT))
        wo_sb = [wo_sb4[:, d, :] for d in range(NMT)]

        # qT/kT: [DL, S] bf16 stored as NMT tiles [128, S]
        qT = [qkv_pool.tile([128, S], BF16, tag=f"qT{m}", name=f"qT{m}") for m in range(NMT)]
        kT = [qkv_pool.tile([128, S], BF16, tag=f"kT{m}", name=f"kT{m}") for m in range(NMT)]

        def proj_chunk(xt, w_sb, bcol, res, qc, m):
            ps = psum_proj.tile([128, 512], F32, tag="proj", name="ps")
            for d in range(NDT):
                nc.tensor.matmul(
                    ps[:], w_sb[d][:, 128 * m:128 * (m + 1)],
                    xt[:, d, :],
                    start=(d == 0), stop=(d == NDT - 1))
            nc.vector.tensor_scalar_add(
                res[m][:, 512 * qc:512 * (qc + 1)], ps[:],
                bqk_sb[:, bcol:bcol + 1])

        # v_aug: per seq-tile [128, HL, 2*HD] bf16; per head [V(64) | ones(64)]
        # so the ctx matmul (M=128, one accumulation group) leaves ctx on psum
        # rows 0-63 and the softmax sums replicated on rows 64-127.  The ones
        # halves are memset once upfront (DVE is idle during the initial DMA).
        v_aug = [qkv_pool.tile([128, HL, 2 * HD], BF16, tag=f"va{st}",
                               name=f"va{st}") for st in range(S // 128)]
        for st in range(S // 128):
            nc.vector.memset(v_aug[st][:, :, HD:2 * HD], 1.0)

        def v_chunk(xt, qc, sti):
            st = 4 * qc + sti
            ps = psum_proj.tile([128, 512], F32, tag="proj", name="ps")
            for d in range(NDT):
                nc.tensor.matmul(ps[:], xt[:, d, 128 * sti:128 * (sti + 1)],
                                 wv_sb[d][:], start=(d == 0), stop=(d == NDT - 1))
            nc.vector.tensor_add(
                v_aug[st][:, :, 0:HD],
                ps[:].rearrange("p (h c) -> p h c", h=HL),
                bvb_sb[:].rearrange("p (h c) -> p h c", h=HL))

        # ---- out projection ----
        ctxT = [qkv_pool.tile([128, S], BF16, tag=f"ctxT{m}", name=f"ctxT{m}") for m in range(NMT)]

        def emit_outproj(qt):
            # evacuation via ScalarE Copy (DVE is busier than ScalarE here)
            ot = outst_pool.tile([128, 1024], BF16, tag="ot", name="ot")
            for n in range(2):
                po_ps = psum_proj.tile([128, 512], F32, tag="proj", name="po_ps")
                for d in range(NMT):
                    nc.tensor.matmul(
                        po_ps[:],
                        ctxT[d][:, 128 * qt:128 * (qt + 1)],
                        wo_sb[d][:, 512 * n:512 * (n + 1)],
                        start=(d == 0), stop=(d == NMT - 1))
                nc.scalar.activation(ot[:, 512 * n:512 * (n + 1)], po_ps[:],
                                     mybir.ActivationFunctionType.Copy)
            nc.sync.dma_start(out_ap[128 * qt:128 * (qt + 1), :], ot[:])

        # ---- chunk 0 projections upfront (diagonal starts at kt=0) ----
        for m in range(NMT):
            proj_chunk(xq_c0, wq_sb, m, qT, 0, m)
        for m in range(NMT):
            proj_chunk(xk_c0, wk_sb, NMT + m, kT, 0, m)
        for sti in range(4):
            v_chunk(xv_c0, 0, sti)

        # x prefetch state: chunk qc's x tiles are loaded during chunk qc-1
        xk_next = xv_next = None

        # ---- attention per q-chunk ----
        for qc in range(NQC):
            xk_cur, xv_cur = xk_next, xv_next
            nkt = 4 * (qc + 1)
            ndiag = 4 * qc
            G = (HL // 2) * nkt  # global iteration count for this chunk

            # filler list: (deadline_g, closure); sorted by deadline.
            # Deadlines are emission-order iterations g = hp*nkt + kt by which
            # the closure must have been emitted (data-dependency order).
            items = []
            BIG = 10 ** 6
            if qc > 0:
                # this chunk's v tiles (st >= 1): ctx(kt=ndiag+s) emitted at
                # python iteration ndiag+s+4 (pend depth); keep margin 2
                for s in range(1, 4):
                    items.append((min(ndiag + s + 2, nkt),
                                  lambda s=s, x=xv_cur: v_chunk(x, qc, s)))
                # this chunk's kT m-groups (m >= 1): scores(kt=ndiag) of pair
                # m emitted at g = m*nkt + ndiag; margin 2
                for m in range(1, NMT):
                    items.append((m * nkt + ndiag - 2,
                                  lambda m=m, x=xk_cur: proj_chunk(x, wk_sb, NMT + m, kT, qc, m)))
            if qc + 1 < NQC:
                xq_c = load_xt_chunk(xq_ap, qc + 1, "q")
                xk_next = load_xt_chunk(xk_ap, qc + 1, "k")
                xv_next = load_xt_chunk(xv_ap, qc + 1, "v")
                for m in range(NMT):
                    items.append((BIG, lambda m=m, x=xq_c, q=qc + 1: proj_chunk(x, wq_sb, m, qT, q, m)))
                # hoist next chunk's first kT / v group into this chunk
                items.append((BIG, lambda x=xk_next, q=qc + 1: proj_chunk(x, wk_sb, NMT + 0, kT, q, 0)))
                items.append((BIG, lambda x=xv_next, q=qc + 1: v_chunk(x, q, 0)))
            if qc > 0:
                items += [(BIG, lambda qt=4 * (qc - 1) + j: emit_outproj(qt))
                          for j in range(4)]
            items.sort(key=lambda it: it[0])
            nit = len(items)
            ndone = 0

            for hp in range(HL // 2):
                heads = (2 * hp, 2 * hp + 1)
                ctx_ps = {h: psum_ctx.tile([128, 512], F32, tag="ctx",
                                           name=f"ctx{h}") for h in heads}

                def emit_scores_exp(kt):
                    qs = max(0, 128 * kt - 512 * qc)
                    sc_ps = psum_big.tile([128, 1024], F32, tag="big", name="sc")
                    for i in range(2):
                        po = 64 * i
                        nc.tensor.matmul(
                            sc_ps[:, 512 * i + qs:512 * (i + 1)],
                            kT[hp][po:po + HD, 128 * kt:128 * (kt + 1)],
                            qT[hp][po:po + HD, 512 * qc + qs:512 * (qc + 1)],
                            start=True, stop=True)
                    # late chunks are Scalar(exp)-bound: offload a slice of
                    # non-diagonal tiles to DVE via the exp bit-trick
                    offload = (qc == 2 and kt % 4 == 3 and kt < ndiag) or \
                              (qc == 3 and kt % 3 == 2 and kt < ndiag)
                    if offload:
                        it = sch_pool.tile([128, 1024], mybir.dt.int16,
                                           tag="it", name="it")
                        nc.vector.tensor_scalar(
                            it[:], sc_ps[:], SK16, SB16,
                            mybir.AluOpType.mult, mybir.AluOpType.add)
                        return it[:].bitcast(BF16)
                    et = exp_pool.tile([128, 1024], BF16, tag="et", name="et")
                    nc.scalar.activation(et[:, qs:1024], sc_ps[:, qs:1024],
                                         EXP, scale=SCALE)
                    return et

                def emit_ctx(kt, et):
                    qs = max(0, 128 * kt - 512 * qc)
                    diag = ndiag <= kt
                    for i, h in enumerate(heads):
                        if diag:  # mask k>q in the diagonal 128x128 block
                            nc.gpsimd.affine_select(
                                out=et[:, 512 * i + qs:512 * i + qs + 128],
                                in_=et[:, 512 * i + qs:512 * i + qs + 128],
                                compare_op=mybir.AluOpType.is_ge, fill=0.0,
                                base=0, pattern=[[1, 128]], channel_multiplier=-1)
                        nc.tensor.matmul(
                            ctx_ps[h][:, qs:512],
                            v_aug[kt][:, h, :],
                            et[:, 512 * i + qs:512 * (i + 1)],
                            start=(kt == 0), stop=(kt == nkt - 1))

                # software pipeline: scores/exp run ahead of ctx
                pend = []
                for kt in range(nkt):
                    pend.append((kt, emit_scores_exp(kt)))
                    if len(pend) > 4:
                        emit_ctx(*pend.pop(0))
                    # fillers: emit when due (deadline) or by uniform pacing
                    g = hp * nkt + kt + 1
                    want = (nit * g) // G
                    while ndone < nit and (ndone < want or items[ndone][0] <= g):
                        items[ndone][1]()
                        ndone += 1
                # force items due within this pair before draining ctx
                lim = (hp + 1) * nkt
                while ndone < nit and items[ndone][0] <= lim:
                    items[ndone][1]()
                    ndone += 1
                for pn in pend:
                    emit_ctx(*pn)

                # normalization: replicated sums (psum rows 64-127) -> SBUF
                # base 0 (the custom recip op requires base-0 operands),
                # reciprocal, then multiply straight out of PSUM
                for h in heads:
                    po = 64 * (h % 2)
                    sums = norm_pool.tile([HD, 512], F32, tag="sums", name="sums")
                    nc.vector.tensor_copy(sums[:], ctx_ps[h][64:128, :])
                    recip = norm_pool.tile([HD, 512], F32, tag="recip", name="recip")
                    nc.vector.reciprocal_approx_fast(recip[:], sums[:])
                    nc.vector.tensor_mul(
                        ctxT[hp][po:po + HD, 512 * qc:512 * (qc + 1)],
                        ctx_ps[h][0:HD, :], recip[:])

            while ndone < nit:
                items[ndone][1]()
                ndone += 1

        for qt in range(4 * (NQC - 1), 4 * NQC):
            emit_outproj(qt)

    nc.compile()
    return nc


def _shard(inputs):
    in_maps = []
    for c in range(N_CORES):
        b, g = c // 2, c % 2
        sl = slice(512 * g, 512 * (g + 1))
        wqkv = np.concatenate([
            inputs["Wq"][:, sl], inputs["Wk"][:, sl], inputs["Wv"][:, sl]],
            axis=0).astype(ml_dtypes.bfloat16)
        bqk = np.empty((128, 8), np.float32)
        for m in range(4):
            bqk[:, m] = inputs["bq"][sl][128 * m:128 * (m + 1)]
            bqk[:, 4 + m] = inputs["bk"][sl][128 * m:128 * (m + 1)]
        in_maps.append({
            "xqt": np.ascontiguousarray(inputs["inputs_q"][b].T.astype(ml_dtypes.bfloat16)),
            "xkt": np.ascontiguousarray(inputs["inputs_k"][b].T.astype(ml_dtypes.bfloat16)),
            "xvt": np.ascontiguousarray(inputs["inputs_v"][b].T.astype(ml_dtypes.bfloat16)),
            "wqkv": np.ascontiguousarray(wqkv),
            "bqk": bqk,
            "bvb": np.ascontiguousarray(
                np.broadcast_to(inputs["bv"][sl], (128, 512))).astype(np.float32),
            "wo": np.ascontiguousarray(inputs["Wo"][sl, :].astype(ml_dtypes.bfloat16)),
        })
    return in_maps


def kernel(**inputs):
    global _compiled
    inputs = {k: np.asarray(v, dtype=np.float32) for k, v in inputs.items()}
    if _compiled is None:
        _compiled = _build()
    nc = _compiled
    in_maps = _shard(inputs)
    res = run_bass_kernel_spmd(nc, in_maps, list(range(N_CORES)),
                               trace=bool(int(__import__("os").environ.get("BASS_TRACE", "0"))))
    kernel.last_results = res
    B = 4
    out = np.empty((B, S, D), np.float32)
    for b in range(B):
        out[b] = (res.results[2 * b]["out"].astype(np.float32)
                  + res.results[2 * b + 1]["out"].astype(np.float32))
    out += inputs["bo"][None, None, :]
    return out
